# revision 25
# baseline (speedup 1.0000x reference)
"""Multi-head causal attention (B=2, S=2048, D=1024, H=16) on 8 trn2 cores.

Sharding: core c handles batch b = c // 4 and head group g = c % 4 (4 heads,
256 feature columns). Each core computes its heads' attention context and a
partial output projection (ctx_g @ Wo[rows_g]); the host sums the 4 partials
per batch and adds bo.

Per-core kernel layout choices (all matmuls in fp32r):
- x is pre-transposed on the host to xT [D, S] so the contraction dim (d) of
  the QKV projections sits on SBUF partitions with no on-device transposes.
- Q^T, K^T [256, S] are produced head-major so scores can be computed in
  transposed layout S^T[sk, sq] = K @ Q^T; then P^T = exp(S^T) is directly the
  moving operand of ctx^T = (V|1)^T.T @ P^T, so flash-style PV needs no
  transpose either.
- Softmax: scores/8 are small (|s|<~3), so exp without max subtraction is
  safe; the denominator comes from a ones column folded into the V stationary
  operand; normalization multiplies ctx^T by a DMA-broadcast reciprocal row.
- Causal mask: gpsimd affine_select zeroes p^T entries with sk > sq on the 4
  diagonal tiles of each (head, sq-tile); fully-masked tiles are skipped.
"""

import os
import sys
import types
from contextlib import ExitStack

import numpy as np

import concourse.bacc as bacc
import concourse.bass as bass
import concourse.mybir as mybir
import concourse.tile as tile
from concourse.bass_utils import run_bass_kernel_spmd


def _install_ntff_hook():
    """The agent image's antenv lacks axon_hooks, so trn_boot's NTFF hook
    install degrades silently. Recreate the module + hook so trace=True works."""
    if "antenv.axon_hooks" in sys.modules:
        return
    try:
        mod = types.ModuleType("antenv.axon_hooks")
        holder = [None]
        mod.set_axon_ntff_profile_hook = lambda h: holder.__setitem__(0, h)
        mod.get_axon_ntff_profile_hook = lambda: holder[0]
        from trn_agent_boot.trn_boot import _ntff_profile_via_ctypes

        hook = _ntff_profile_via_ctypes("/opt/axon/libaxon_pjrt.so")
        if hook is None:
            return
        mod.set_axon_ntff_profile_hook(hook)
        sys.modules["antenv.axon_hooks"] = mod
    except Exception:
        pass

B, S, D, H, HD = 2, 2048, 1024, 16, 64
NCORES = 8
GROUPS = 4          # head groups (cores) per batch
HC = H // GROUPS    # heads per core
DG = HC * HD        # feature columns per core (256)
P = 128
KSUB = D // P       # 8 contraction subtiles for the projections
SQT = 512           # sq tile width (free dim of scores/ctx matmuls)
NSQ = S // SQT      # 4
NST = S // P        # 16 s subtiles of 128
F32 = mybir.dt.float32
F32R = mybir.dt.float32r

_CACHE = {}


def _mha_tile_kernel(tc, xT, wq, wk, wv, wo, out):
    nc = tc.nc
    scale = 1.0 / np.sqrt(np.float32(HD))

    with ExitStack() as ctx:
        consts = ctx.enter_context(tc.tile_pool(name="consts", bufs=1))
        dramp = ctx.enter_context(tc.tile_pool(name="dramp", bufs=3, space="DRAM"))
        # PSUM: two 2-bank [128,1024] working tiles + four 1-bank ctx accumulators
        sps = ctx.enter_context(tc.tile_pool(name="sps", bufs=2, space="PSUM"))
        cps = ctx.enter_context(tc.tile_pool(name="cps", bufs=4, space="PSUM"))
        # x slices + rotating QKV weights; released after the projections so
        # the attention-phase pools reuse the space
        xw = tc.alloc_tile_pool(name="xw", bufs=1)

        # --- persistent SBUF tensors ---
        wo_sb = consts.tile([P, DG // P, D], F32R)
        wq_sb = xw.tile([P, KSUB, DG], F32R, tag="w", bufs=3, name="wq_sb")
        wk_sb = xw.tile([P, KSUB, DG], F32R, tag="w", bufs=3, name="wk_sb")
        wv_sb = xw.tile([P, KSUB, DG], F32R, tag="w", bufs=3, name="wv_sb")
        nc.sync.dma_start(out=wq_sb, in_=wq)

        qt_sb = consts.tile([P, DG // P, S], F32R)   # Q^T: head h at [64*(h%2):, h//2, :]
        # K^T zero-padded per head: head h's 64 rows live at [64*(h%2):, h, :],
        # the other 64 rows are 0 so score matmuls contract over K=128 (keeps
        # the PE's HAM activity monitor engaged at full clock).
        kt_sb = consts.tile([P, HC, S], F32R)
        # V with the ones column baked in, per s-subtile and head:
        #   even h: [V(64) | 1 | 0(63)]  -> ctx rows 0-63, denom row 64
        #   odd  h: [1 | 0(63) | V(64)]  -> denom row 0, ctx rows 64-127
        v_sb = consts.tile([P, NST, HC, P], F32R)
        ctxt_sb = consts.tile([P, DG // P, S], F32R)  # normalized ctx^T, same layout as qt

        # memset can't write fp32r; broadcast-copy from small f32 scratch instead
        zsc = consts.tile([P, P], F32, tag="zsc", bufs=1)
        nc.vector.memset(zsc, 0.0)
        osc = consts.tile([P, 1], F32, tag="osc", bufs=1)
        nc.vector.memset(osc, 1.0)
        nc.vector.tensor_copy(
            out=v_sb, in_=zsc[:, None, None, :].to_broadcast((P, NST, HC, P))
        )
        nc.vector.tensor_copy(
            out=kt_sb.rearrange("p h (a b) -> p h a b", b=P),
            in_=zsc[:, None, None, :].to_broadcast((P, HC, S // P, P)),
        )
        for h in range(HC):
            ones_col = 64 if h % 2 == 0 else 0
            nc.vector.tensor_copy(
                out=v_sb[:, :, h, ones_col : ones_col + 1],
                in_=osc[:, None, :].to_broadcast((P, NST, 1)),
            )

        # --- phase 1+2: stream xT by sq-slice; QT/KT/V interleaved per slice
        # so attention tiles unblock as soon as slice 0 is projected.
        for n in range(NSQ):
            nsl = slice(n * SQT, (n + 1) * SQT)
            xn = xw.tile([P, KSUB, SQT], F32R, tag="xT", bufs=3, name=f"xn_{n}")
            for k in range(KSUB):
                nc.sync.dma_start(
                    out=xn[:, k, :], in_=xT[k * P : (k + 1) * P, n * SQT : (n + 1) * SQT]
                )
            if n == 0:
                # needed only from the KT/V chains on; keep them behind slice 0
                nc.sync.dma_start(out=wk_sb, in_=wk)
                nc.sync.dma_start(out=wv_sb, in_=wv)
                nc.scalar.dma_start(out=wo_sb, in_=wo)
            ps = sps.tile([P, 2 * SQT], F32, tag="s", name=f"qps_{n}")
            for m in range(DG // P):
                for k in range(KSUB):
                    nc.tensor.matmul(
                        ps[:, m * SQT : (m + 1) * SQT],
                        lhsT=wq_sb[:, k, m * P : (m + 1) * P],
                        rhs=xn[:, k, :],
                        start=(k == 0),
                        stop=(k == KSUB - 1),
                    )
            nc.vector.tensor_copy(
                out=qt_sb[:, :, nsl],
                in_=ps.rearrange("p (m f) -> p m f", f=SQT),
            )
            ps = sps.tile([P, 2 * SQT], F32, tag="s", name=f"kps_{n}")
            for m in range(DG // P):
                for k in range(KSUB):
                    nc.tensor.matmul(
                        ps[:, m * SQT : (m + 1) * SQT],
                        lhsT=wk_sb[:, k, m * P : (m + 1) * P],
                        rhs=xn[:, k, :],
                        start=(k == 0),
                        stop=(k == KSUB - 1),
                    )
            psv = ps.rearrange("p (m f) -> p m f", f=SQT)
            nc.vector.tensor_copy(out=kt_sb[0:64, 0::2, nsl], in_=psv[0:64, :, :])
            nc.vector.tensor_copy(out=kt_sb[64:P, 1::2, nsl], in_=psv[64:P, :, :])
            ps = sps.tile([P, 2 * SQT], F32, tag="s", name=f"vps_{n}")
            for sst in range(SQT // P):
                for k in range(KSUB):
                    nc.tensor.matmul(
                        ps[:, sst * DG : (sst + 1) * DG],
                        lhsT=xn[:, k, sst * P : (sst + 1) * P],
                        rhs=wv_sb[:, k, :],
                        start=(k == 0),
                        stop=(k == KSUB - 1),
                    )
            st0 = n * (SQT // P)
            # psum view: [128, st(4), h(4), 64]; even heads -> cols 0:64,
            # odd heads -> cols 64:128 of the padded V layout
            psv = ps.rearrange("p (t h d) -> p t h d", h=HC, d=HD)
            nc.vector.tensor_copy(
                out=v_sb[:, st0 : st0 + 4, 0:HC:2, 0:HD], in_=psv[:, :, 0:HC:2, :]
            )
            nc.vector.tensor_copy(
                out=v_sb[:, st0 : st0 + 4, 1:HC:2, HD:P], in_=psv[:, :, 1:HC:2, :]
            )

        xw.release()
        ptp = ctx.enter_context(tc.tile_pool(name="ptp", bufs=6))
        smalls = ctx.enter_context(tc.tile_pool(name="smalls", bufs=3))
        outp = ctx.enter_context(tc.tile_pool(name="outp", bufs=3))

        # --- phase 3: attention, sk-tile-major; the up-to-4 sq-tiles per
        # sk-tile are independent chains that keep the PE dense. Scores/exp/PV
        # windowed to valid columns [w0:512]; sq-tile pairs share one 2-bank
        # psum tile so exp runs as one wide ACTIVATE.
        for h in range(HC):
            hm = h // 2
            hp = 64 * (h % 2)
            ctx_rows = 0 if h % 2 == 0 else 64
            denom_row = 64 if h % 2 == 0 else 0
            cpsums = [
                cps.tile([P, SQT], F32, tag="ctx", name=f"ctx_{h}_{i}")
                for i in range(NSQ)
            ]
            for ski in range(NST):
                sqts = list(range(ski // 4, NSQ))
                pts = []
                for pair0 in range(0, len(sqts), 2):
                    grp = sqts[pair0 : pair0 + 2]
                    spsum = sps.tile(
                        [P, 2 * SQT], F32, tag="s", name=f"s_{h}_{ski}_{pair0}"
                    )
                    pt = ptp.tile(
                        [P, 2 * SQT], F32R, tag="pt", name=f"pt_{h}_{ski}_{pair0}"
                    )
                    w0g = None
                    for jj, sqt in enumerate(grp):
                        sq0 = sqt * SQT
                        diag = ski >= 4 * sqt
                        w0 = (128 * ski - sq0) if diag else 0
                        if w0g is None:
                            w0g = jj * SQT + w0
                        base = jj * SQT
                        nc.tensor.matmul(
                            spsum[:, base + w0 : base + SQT],
                            lhsT=kt_sb[:, h, ski * P : (ski + 1) * P],
                            rhs=qt_sb[:, hm, sq0 + w0 : sq0 + SQT],
                            start=True,
                            stop=True,
                        )
                        pts.append((sqt, w0, pt, base, diag))
                    wend = (len(grp) - 1) * SQT + SQT
                    nc.scalar.activation(
                        out=pt[:, w0g:wend], in_=spsum[:, w0g:wend],
                        func=mybir.ActivationFunctionType.Exp,
                        bias=0.0, scale=float(scale),
                    )
                for sqt, w0, pt, base, diag in pts:
                    if diag:  # zero entries with sk > sq in the triangular block
                        nc.gpsimd.affine_select(
                            out=pt[:, base + w0 : base + w0 + P],
                            in_=pt[:, base + w0 : base + w0 + P],
                            pattern=[[1, P]],
                            compare_op=mybir.AluOpType.is_ge,
                            fill=0.0,
                            base=0,
                            channel_multiplier=-1,
                        )
                for sqt, w0, pt, base, diag in pts:
                    nc.tensor.matmul(
                        cpsums[sqt][:, w0:],
                        lhsT=v_sb[:, ski, h, :],
                        rhs=pt[:, base + w0 : base + SQT],
                        start=(ski == 0),
                        stop=(ski == 4 * sqt + 3),
                    )
                    if ski == 4 * sqt + 3:
                        # normalize eagerly once this sq-tile's chain stops:
                        # ctx rows *= 1/denom (broadcast across partitions).
                        sq0 = sqt * SQT
                        cpsum = cpsums[sqt]
                        rec_t = smalls.tile([P, SQT], F32, tag="recip")
                        nc.vector.tensor_copy(
                            out=rec_t[denom_row : denom_row + 1, :],
                            in_=cpsum[denom_row : denom_row + 1, :],
                        )
                        # partition-scatter so reciprocal uses all DVE lanes
                        spread = smalls.tile([P, SQT // P], F32, tag="spread")
                        nc.sync.dma_start(
                            out=spread, in_=rec_t[denom_row : denom_row + 1, :]
                        )
                        nc.vector.reciprocal(out=spread, in_=spread)
                        rec_d2 = dramp.tile([1, SQT], F32, tag="rec_d2")
                        nc.sync.dma_start(
                            out=rec_d2.rearrange("a (p f) -> (a p) f", p=P),
                            in_=spread,
                        )
                        bcast = smalls.tile([P, SQT], F32, tag="bcast")
                        rec_b = bass.AP(
                            tensor=rec_d2.tensor,
                            offset=rec_d2.offset,
                            ap=[[0, 64]] + [list(p) for p in rec_d2.ap[1:]],
                        )
                        nc.sync.dma_start(
                            out=bcast[ctx_rows : ctx_rows + 64, :], in_=rec_b
                        )
                        nc.vector.tensor_tensor(
                            ctxt_sb[hp : hp + 64, hm, sq0 : sq0 + SQT],
                            cpsum[ctx_rows : ctx_rows + 64, :],
                            bcast[ctx_rows : ctx_rows + 64, :],
                            mybir.AluOpType.mult,
                        )

        # --- phase 4: partial output projection out = ctx @ Wo_slice ---
        for st in range(NST):
            ot = outp.tile([P, D], F32, tag="out")
            for nn in range(D // SQT):
                ps = cps.tile([P, SQT], F32, tag="ctx", name=f"ops_{st}_{nn}")
                for k in range(DG // P):
                    nc.tensor.matmul(
                        ps,
                        lhsT=ctxt_sb[:, k, st * P : (st + 1) * P],
                        rhs=wo_sb[:, k, nn * SQT : (nn + 1) * SQT],
                        start=(k == 0),
                        stop=(k == DG // P - 1),
                    )
                nc.scalar.copy(out=ot[:, nn * SQT : (nn + 1) * SQT], in_=ps)
            nc.sync.dma_start(out=out[st * P : (st + 1) * P, :], in_=ot)


def build_nc():
    if "nc" in _CACHE:
        return _CACHE["nc"]
    nc = bacc.Bacc("TRN2", target_bir_lowering=False, debug=False, num_devices=NCORES)
    xT = nc.dram_tensor("xT", (D, S), F32R, kind="ExternalInput").ap()
    wq = nc.dram_tensor("wq", (P, KSUB, DG), F32R, kind="ExternalInput").ap()
    wk = nc.dram_tensor("wk", (P, KSUB, DG), F32R, kind="ExternalInput").ap()
    wv = nc.dram_tensor("wv", (P, KSUB, DG), F32R, kind="ExternalInput").ap()
    wo = nc.dram_tensor("wo", (P, DG // P, D), F32R, kind="ExternalInput").ap()
    out = nc.dram_tensor("out", (S, D), F32, kind="ExternalOutput").ap()
    with tile.TileContext(nc) as tc:
        _mha_tile_kernel(tc, xT, wq, wk, wv, wo, out)
    nc.compile()
    _CACHE["nc"] = nc
    return nc


def make_in_maps(x, Wq, Wk, Wv, Wo):
    x = np.asarray(x, np.float32)
    in_maps = []
    for c in range(NCORES):
        b, g = c // GROUPS, c % GROUPS
        cols = slice(g * DG, (g + 1) * DG)

        def wslice(W):
            # [D, DG] -> [128, KSUB, DG] with [p, k, m] = W[k*128+p, m]
            return np.ascontiguousarray(
                np.asarray(W, np.float32)[:, cols].reshape(KSUB, P, DG).transpose(1, 0, 2)
            )

        wo_c = np.ascontiguousarray(
            np.asarray(Wo, np.float32)[cols, :].reshape(DG // P, P, D).transpose(1, 0, 2)
        )
        in_maps.append(
            {
                "xT": np.ascontiguousarray(x[b].T),
                "wq": wslice(Wq),
                "wk": wslice(Wk),
                "wv": wslice(Wv),
                "wo": wo_c,
            }
        )
    return in_maps


def kernel(x, Wq, Wk, Wv, Wo, bo):
    nc = build_nc()
    in_maps = make_in_maps(x, Wq, Wk, Wv, Wo)
    trace = bool(int(os.environ.get("MHA_TRACE", "0")))
    if trace:
        _install_ntff_hook()
    res = run_bass_kernel_spmd(
        nc, in_maps, core_ids=list(range(NCORES)), trace=trace,
        trace_cores=list(range(NCORES)) if trace else None,
    )
    _CACHE["last_results"] = res
    bo = np.asarray(bo, np.float32)
    out = np.zeros((B, S, D), np.float32)
    for c in range(NCORES):
        out[c // GROUPS] += res.results[c]["out"]
    out += bo[None, None, :]
    return out
